# revision 3
# baseline (speedup 1.0000x reference)
"""Grouped GEMM (MoE routing) Trainium2 kernel.

Full inputs in, full output out. Strategy: tensor-parallel shard of the
output N dimension across the 8 NeuronCores (each core computes all tokens
x a 512-column slice of N). The host pre-transposes operands so both
matmul operands stream K on SBUF partitions, and pads each token segment
to a multiple of 128 in a reordered activation layout, so every 128-token
tile maps to exactly one expert (the host-known segment schedule is baked
into the instruction stream). Matmuls run in float32r mode (full fp32
operands, 1 cycle/row at free-dim 512). Padded output rows are dropped on
the host.
"""

import os
import sys
from contextlib import ExitStack

import numpy as np

for _p in ("/opt/trn_rl_repo", "/root/.axon_site/_ro/trn_rl_repo"):
    if os.path.isdir(_p) and _p not in sys.path:
        sys.path.insert(0, _p)

import concourse.bass as bass  # noqa: E402,F401
import concourse.tile as tile  # noqa: E402
from concourse import bacc, mybir  # noqa: E402

E, T, K, N = 8, 8192, 2048, 4096
NCORES = 8
NS = N // NCORES  # output columns per core
P = 128
KB = K // P  # contraction blocks
LAST_RESULT = None  # BassKernelResults of the most recent run (for test.py)


def _dtypes():
    kind = os.environ.get("KERNEL_DTYPE", "f32r")
    if kind == "bf16":
        import ml_dtypes

        return mybir.dt.bfloat16, ml_dtypes.bfloat16
    if kind == "f32":
        return mybir.dt.float32, np.float32
    return mybir.dt.float32r, np.float32


def _padded_layout(seg):
    """Pad each segment to a 128-multiple.

    Returns (tile_slots, tok_pos): tile_slots[i] = segment slot for padded
    tile i; tok_pos[t] = padded row index of original token t.
    """
    tile_slots = []
    tok_pos = np.empty(T, dtype=np.int64)
    for s in range(E):
        lo, hi = int(seg[s]), int(seg[s + 1])
        if hi == lo:
            continue
        start = len(tile_slots) * P
        tok_pos[lo:hi] = start + np.arange(hi - lo)
        tile_slots.extend([s] * (-(-(hi - lo) // P)))
    return tile_slots, tok_pos


def _build(tile_slots, mm_dt):
    tp = len(tile_slots) * P  # padded token count
    nc = bacc.Bacc(
        "TRN2",
        target_bir_lowering=False,
        debug=False,
        enable_asserts=False,
        num_devices=NCORES,
    )
    at = nc.dram_tensor("at", [K, tp], mm_dt, kind="ExternalInput").ap()
    bt = nc.dram_tensor("bt", [E, K, NS], mm_dt, kind="ExternalInput").ap()
    out = nc.dram_tensor("out", [tp, NS], mybir.dt.float32, kind="ExternalOutput").ap()

    with tile.TileContext(nc) as tc, ExitStack() as ctx:
        wpool = ctx.enter_context(tc.tile_pool(name="w", bufs=3))
        apool = ctx.enter_context(tc.tile_pool(name="a", bufs=6))
        pspool = ctx.enter_context(tc.tile_pool(name="ps", bufs=4, space="PSUM"))
        opool = ctx.enter_context(tc.tile_pool(name="o", bufs=4))

        w_tiles = {}

        def get_w(s):
            if s not in w_tiles:
                wt = wpool.tile([P, KB, NS], mm_dt, tag="w")
                nc.sync.dma_start(
                    out=wt[:], in_=bt[s].rearrange("(kb p) n -> p kb n", p=P)
                )
                w_tiles[s] = wt
            return w_tiles[s]

        for i, s in enumerate(tile_slots):
            atile = apool.tile([P, KB, P], mm_dt, tag="a")
            nc.sync.dma_start(
                out=atile[:],
                in_=at[:, i * P : (i + 1) * P].rearrange("(kb p) m -> p kb m", p=P),
            )
            wt = get_w(s)
            ps = pspool.tile([P, NS], mybir.dt.float32, tag="ps")
            for kb in range(KB):
                nc.tensor.matmul(
                    ps[:, :],
                    lhsT=atile[:, kb, :],
                    rhs=wt[:, kb, :],
                    start=(kb == 0),
                    stop=(kb == KB - 1),
                )
            ot = opool.tile([P, NS], mybir.dt.float32, tag="o")
            nc.vector.tensor_copy(ot[:], ps[:])
            nc.sync.dma_start(out=out[i * P : (i + 1) * P, :], in_=ot[:])

    nc.compile()
    return nc


def kernel(a, b, c, batch_size, weight_column_major, seg_indptr, weight_indices, **_):
    from concourse.bass_utils import run_bass_kernel_spmd

    global LAST_RESULT
    mm_dt, np_dt = _dtypes()

    a = np.asarray(a, dtype=np.float32)
    b = np.asarray(b, dtype=np.float32)
    seg = [int(x) for x in np.asarray(seg_indptr)]
    widx = [int(x) for x in np.asarray(weight_indices)]

    tile_slots, tok_pos = _padded_layout(seg)
    tp = len(tile_slots) * P

    # Padded, transposed activations: at_pad[:, tok_pos[t]] = a[t, :]
    at_pad = np.zeros((K, tp), dtype=np_dt)
    aT = np.ascontiguousarray(a.T).astype(np_dt, copy=False)
    for s in range(E):
        lo, hi = seg[s], seg[s + 1]
        if hi > lo:
            at_pad[:, tok_pos[lo] : tok_pos[lo] + (hi - lo)] = aT[:, lo:hi]

    bperm = b[widx]  # [E, N, K] in segment-slot order
    in_maps = []
    for cidx in range(NCORES):
        btc = np.ascontiguousarray(
            np.swapaxes(bperm[:, cidx * NS : (cidx + 1) * NS, :], 1, 2)
        ).astype(np_dt, copy=False)  # [E, K, NS]
        in_maps.append({"at": at_pad, "bt": btc})

    nc = _build(tile_slots, mm_dt)
    trace = bool(int(os.environ.get("KERNEL_TRACE", "0")))
    tmpdir = None
    if trace:
        import shutil

        tmpdir = os.environ.get("KERNEL_TRACE_DIR", "/tmp/ntff_out")
        shutil.rmtree(tmpdir, ignore_errors=True)
        os.makedirs(tmpdir, exist_ok=True)
    res = run_bass_kernel_spmd(
        nc,
        in_maps,
        core_ids=list(range(NCORES)),
        trace=trace,
        tmpdir=tmpdir,
    )
    LAST_RESULT = res

    out_pad = np.empty((tp, N), dtype=np.float32)
    for cidx in range(NCORES):
        out_pad[:, cidx * NS : (cidx + 1) * NS] = res.results[cidx]["out"]
    return out_pad[tok_pos]


# revision 5
# speedup vs baseline: 1.9379x; 1.9379x over previous
"""Grouped GEMM (MoE routing) Trainium2 kernel.

Full inputs in, full output out. Strategy: tensor-parallel shard of the
output N dimension across the 8 NeuronCores (each core computes all tokens
x a 512-column slice of N). The host pre-transposes operands so both
matmul operands stream K on SBUF partitions, and pads each token segment
to a multiple of 128 in a reordered activation layout, so every 128-token
tile maps to exactly one expert (the host-known segment schedule is baked
into the instruction stream). Matmuls run in float32r mode (full fp32
operands, 1 cycle/row at free-dim 512). Padded output rows are dropped on
the host.
"""

import os
import sys
from contextlib import ExitStack

import numpy as np

for _p in ("/opt/trn_rl_repo", "/root/.axon_site/_ro/trn_rl_repo"):
    if os.path.isdir(_p) and _p not in sys.path:
        sys.path.insert(0, _p)

import concourse.bass as bass  # noqa: E402,F401
import concourse.tile as tile  # noqa: E402
from concourse import bacc, mybir  # noqa: E402

E, T, K, N = 8, 8192, 2048, 4096
NCORES = 8
NS = N // NCORES  # output columns per core
P = 128
KB = K // P  # contraction blocks
LAST_RESULT = None  # BassKernelResults of the most recent run (for test.py)


def _dtypes():
    kind = os.environ.get("KERNEL_DTYPE", "f32r")
    if kind == "bf16":
        import ml_dtypes

        return mybir.dt.bfloat16, ml_dtypes.bfloat16
    if kind == "f32":
        return mybir.dt.float32, np.float32
    return mybir.dt.float32r, np.float32


def _padded_layout(seg):
    """Pad each segment to a 128-multiple.

    Returns (tile_slots, tok_pos): tile_slots[i] = segment slot for padded
    tile i; tok_pos[t] = padded row index of original token t.
    """
    tile_slots = []
    tok_pos = np.empty(T, dtype=np.int64)
    for s in range(E):
        lo, hi = int(seg[s]), int(seg[s + 1])
        if hi == lo:
            continue
        start = len(tile_slots) * P
        tok_pos[lo:hi] = start + np.arange(hi - lo)
        tile_slots.extend([s] * (-(-(hi - lo) // P)))
    return tile_slots, tok_pos


def _build(tile_slots, mm_dt):
    tp = len(tile_slots) * P  # padded token count
    nc = bacc.Bacc(
        "TRN2",
        target_bir_lowering=False,
        debug=False,
        enable_asserts=False,
        num_devices=NCORES,
    )
    # Pre-tiled SBUF-native layouts (one contiguous run per partition line):
    # at[i, p, kb, m] = a_padded[i*128 + m, kb*128 + p]
    # bt[s, p, kb, n] = b[widx[s], n_off + n, kb*128 + p]
    at = nc.dram_tensor("at", [tp // P, P, KB, P], mm_dt, kind="ExternalInput").ap()
    bt = nc.dram_tensor("bt", [E, P, KB, NS], mm_dt, kind="ExternalInput").ap()
    out = nc.dram_tensor("out", [tp, NS], mybir.dt.float32, kind="ExternalOutput").ap()

    with tile.TileContext(nc) as tc, ExitStack() as ctx:
        wpool = ctx.enter_context(tc.tile_pool(name="w", bufs=3))
        apool = ctx.enter_context(tc.tile_pool(name="a", bufs=6))
        pspool = ctx.enter_context(tc.tile_pool(name="ps", bufs=4, space="PSUM"))
        opool = ctx.enter_context(tc.tile_pool(name="o", bufs=4))

        w_tiles = {}

        def get_w(s):
            if s not in w_tiles:
                wt = wpool.tile([P, KB, NS], mm_dt, tag="w")
                nc.sync.dma_start(out=wt[:], in_=bt[s])
                w_tiles[s] = wt
            return w_tiles[s]

        for i, s in enumerate(tile_slots):
            atile = apool.tile([P, KB, P], mm_dt, tag="a")
            nc.sync.dma_start(out=atile[:], in_=at[i])
            wt = get_w(s)
            ps = pspool.tile([P, NS], mybir.dt.float32, tag="ps")
            for kb in range(KB):
                nc.tensor.matmul(
                    ps[:, :],
                    lhsT=atile[:, kb, :],
                    rhs=wt[:, kb, :],
                    start=(kb == 0),
                    stop=(kb == KB - 1),
                )
            ot = opool.tile([P, NS], mybir.dt.float32, tag="o")
            nc.vector.tensor_copy(ot[:], ps[:])
            nc.sync.dma_start(out=out[i * P : (i + 1) * P, :], in_=ot[:])

    nc.compile()
    return nc


def kernel(a, b, c, batch_size, weight_column_major, seg_indptr, weight_indices, **_):
    from concourse.bass_utils import run_bass_kernel_spmd

    global LAST_RESULT
    mm_dt, np_dt = _dtypes()

    a = np.asarray(a, dtype=np.float32)
    b = np.asarray(b, dtype=np.float32)
    seg = [int(x) for x in np.asarray(seg_indptr)]
    widx = [int(x) for x in np.asarray(weight_indices)]

    tile_slots, tok_pos = _padded_layout(seg)
    tp = len(tile_slots) * P

    # Padded, transposed activations: at_pad[:, tok_pos[t]] = a[t, :]
    at_pad = np.zeros((K, tp), dtype=np_dt)
    aT = np.ascontiguousarray(a.T).astype(np_dt, copy=False)
    for s in range(E):
        lo, hi = seg[s], seg[s + 1]
        if hi > lo:
            at_pad[:, tok_pos[lo] : tok_pos[lo] + (hi - lo)] = aT[:, lo:hi]
    # -> SBUF-native tiling [ntiles, P(part=k%128), KB, P(m)]
    at_tiled = np.ascontiguousarray(
        at_pad.reshape(KB, P, tp // P, P).transpose(2, 1, 0, 3)
    )

    bperm = b[widx]  # [E, N, K] in segment-slot order
    in_maps = []
    for cidx in range(NCORES):
        btc = np.swapaxes(
            bperm[:, cidx * NS : (cidx + 1) * NS, :], 1, 2
        )  # [E, K, NS] view
        bt_tiled = np.ascontiguousarray(
            btc.reshape(E, KB, P, NS).transpose(0, 2, 1, 3)
        ).astype(np_dt, copy=False)  # [E, P, KB, NS]
        in_maps.append({"at": at_tiled, "bt": bt_tiled})

    nc = _build(tile_slots, mm_dt)
    trace = bool(int(os.environ.get("KERNEL_TRACE", "0")))
    tmpdir = None
    if trace:
        import shutil

        tmpdir = os.environ.get("KERNEL_TRACE_DIR", "/tmp/ntff_out")
        shutil.rmtree(tmpdir, ignore_errors=True)
        os.makedirs(tmpdir, exist_ok=True)
    res = run_bass_kernel_spmd(
        nc,
        in_maps,
        core_ids=list(range(NCORES)),
        trace=trace,
        tmpdir=tmpdir,
    )
    LAST_RESULT = res

    out_pad = np.empty((tp, N), dtype=np.float32)
    for cidx in range(NCORES):
        out_pad[:, cidx * NS : (cidx + 1) * NS] = res.results[cidx]["out"]
    return out_pad[tok_pos]
